# revision 18
# baseline (speedup 1.0000x reference)
"""Trainium2 Bass kernel for nn_CausalSE: causal cumulative-average pooling
+ squeeze-excite gating, data-parallel over batch (one NeuronCore per batch
element).

Reference math per batch element (D=512, T=8192, chunk=16, Tc=512):
    avg    = cumsum(x, t) / (t+1)
    pooled = avg[:, 15::16]                          # [D, Tc]
    h      = relu(w1 @ pooled + b1)                  # [64, Tc]
    g      = sigmoid(w2 @ h + b2)                    # [D, Tc]
    out    = repeat(g, 16, t)[:, :T] * x

The kernel is DMA-bound: 16 MB in + 16 MB out per core at ~435 GB/s DDR is
a ~77 us floor, so everything is scheduled around keeping the DMA rings
saturated end-to-end. Loads all enqueue first on the sync ring (full-rate
prefetch, stores drain behind them FIFO); the gate-multiply outputs feed
two rings (sync + ACT) so neither ring's write path caps the tail.

Compute per 1024-col t-block is sized well under the 4.8 us/block DMA
period: the chunk-sums go through the Tensor engine as q = w1 @ x in
float32r (single-pass fp32 matmul, 1 cycle/row for N>=256 — no cast
needed) so DVE only chunk-reduces the [64, t] PSUM result (16x less
reduce work than reducing x), then scans the causal prefix. The four
in-place gate multiplies split DVE (d0,d1) / GpSimd (d2,d3). x stays
resident in SBUF; total HBM traffic is the 32 MB minimum.
"""

import sys

for _p in ("/opt/trn_rl_repo",):
    if _p not in sys.path:
        sys.path.insert(0, _p)

import numpy as np

B, D, T = 8, 512, 8192
DH = 64          # bottleneck dim = D // 8
CS = 16          # chunksize
TC = T // CS     # 512 chunks
NCORES = 8
NDT = D // 128   # 4 partition tiles of x / out
NB = 8           # t-blocks in the causal pipeline
TB = T // NB     # 1024 cols per block (4 KB DMA rows)
CB = TB // CS    # 64 chunks per block
TH = TB // 2     # 512-col halves: matmul N>=256 keeps float32r at 1 cy/row
CH = TH // CS    # 32 chunks per half

_compiled_nc = None


def build_nc():
    import concourse.tile as tile
    from concourse import bacc, mybir

    f32 = mybir.dt.float32
    f32r = mybir.dt.float32r
    bf16 = mybir.dt.bfloat16
    AF = mybir.ActivationFunctionType
    ALU = mybir.AluOpType
    AX = mybir.AxisListType

    # Bacc (not plain Bass): its finalize() runs the TRN2 sync-wait
    # legalization (move_matmul_waits_to_ldweights / event-semaphore
    # splitting) that walrus codegen requires.
    nc = bacc.Bacc("TRN2", target_bir_lowering=False)
    x_d = nc.declare_dram_parameter("x", [D, T], f32, isOutput=False)
    w1t_d = nc.declare_dram_parameter("w1t", [D, DH], f32, isOutput=False)
    b1_d = nc.declare_dram_parameter("b1", [DH], f32, isOutput=False)
    w2t_d = nc.declare_dram_parameter("w2t", [DH, D], bf16, isOutput=False)
    b2_d = nc.declare_dram_parameter("b2", [D], f32, isOutput=False)
    scale_d = nc.declare_dram_parameter("scale", [DH, TC], f32, isOutput=False)
    out_d = nc.declare_dram_parameter("out", [D, T], f32, isOutput=True)

    with tile.TileContext(nc) as tc:
        with (
            tc.tile_pool(name="xres", bufs=1) as xres,
            tc.tile_pool(name="small", bufs=1) as small,
            tc.tile_pool(name="psum_y", bufs=4, space="PSUM") as psum_y,
            tc.tile_pool(name="psum_g", bufs=4, space="PSUM") as psum_g,
        ):
            xt = xres.tile([128, NDT, T], f32, tag="x")
            gs = small.tile([128, NDT, TC], f32, tag="gs")
            w1s = small.tile([128, NDT, DH], f32, tag="w1")
            w2s = small.tile([DH, D], bf16, tag="w2")
            b1s = small.tile([DH, 1], f32, tag="b1")
            b2s = small.tile([128, NDT], f32, tag="b2")
            scl = small.tile([DH, TC], f32, tag="scl")
            qsum = small.tile([DH, TC], f32, tag="qsum")  # chunk sums of w1@x
            qs = small.tile([DH, TC], f32, tag="qs")      # causal prefix
            h = small.tile([DH, TC], f32, tag="h")
            hb = small.tile([DH, TC], bf16, tag="hb")     # relu(h) for g-mm

            # -- replicated weights / constants, on the ACT queue so the
            # sync ring's 16 MB x prefetch starts at first byte. Only 8
            # DMA-completion semaphores exist globally and each dma_start
            # waits for its semaphore's previous user to COMPLETE, so every
            # DMA instruction saved here lets an x load issue sooner: fuse
            # all weights into 5 DMAs. (w1 dest tagged f32r: the BIR
            # verifier requires FP32r matmult operands from an f32r-typed
            # producer.)
            nc.scalar.dma_start(
                w1s[:].bitcast(f32r),
                w1t_d.rearrange("(dt p) o -> p dt o", p=128).bitcast(f32r),
            )
            nc.scalar.dma_start(w2s[:], w2t_d[:])
            nc.scalar.dma_start(b1s[:], b1_d[:].unsqueeze(1))
            nc.scalar.dma_start(
                b2s[:], b2_d.rearrange("(dt p) -> p dt", p=128)
            )
            nc.scalar.dma_start(scl[:], scale_d[:])

            # All loads enqueue up front on the sync ring: the ring drains
            # FIFO, so loads prefetch at the full rate and the stores
            # appended later fill the ring's tail without starving it.
            # One fused 2 MB DMA per block — separate per-dtile loads issue
            # ~2 us apart (DMA-semaphore recycle latency), which starves
            # the ring and pushes the last block's arrival past 70 us.
            xsrc = x_d.rearrange("(dt p) t -> p dt t", p=128)
            for b in range(NB):
                t0 = b * TB
                nc.sync.dma_start(
                    xt[:, :, t0:t0 + TB].bitcast(f32r),
                    xsrc[:, :, t0:t0 + TB].bitcast(f32r),
                )

            # Causal pipeline, software-pipelined with a 2-block skew so no
            # in-order engine ever waits on same-block results: PE runs
            # q-mm(b) before g-mm(b-2), DVE runs reduce/scan(b) before the
            # gate multiplies of block b-2. Compute then runs ahead of the
            # DMA rings and the store tail is pure ring drain.
            yps = {}

            def stage_q2(b0):
                # y = w1 @ x on PE in float32r for blocks b0, b0+1 as one
                # 16-matmul burst, ki-outer so the stationary only changes
                # 4 times and the PE pipeline stays hot (p-state ramp)
                blks = [b for b in (b0, b0 + 1) if b < NB]
                for b in blks:
                    yps[b] = [
                        psum_y.tile([DH, TH], f32, tag="y", name=f"yp{b}_{hh}")
                        for hh in range(2)
                    ]
                for ki in range(NDT):
                    for b in blks:
                        t0 = b * TB
                        for hh in range(2):
                            nc.tensor.matmul(
                                yps[b][hh][:],
                                w1s[:, ki, :].bitcast(f32r),
                                xt[:, ki, t0 + hh * TH:t0 + (hh + 1) * TH]
                                .bitcast(f32r),
                                start=(ki == 0),
                                stop=(ki == NDT - 1),
                            )

            def stage_prefix(b):
                c0 = b * CB
                yp = yps.pop(b)
                # chunk sums of y: 16x less DVE reduce work than on x
                for hh in range(2):
                    nc.vector.reduce_sum(
                        qsum[:, c0 + hh * CH:c0 + (hh + 1) * CH],
                        yp[hh][:].rearrange("p (c j) -> p c j", j=CS),
                        axis=AX.X,
                    )
                # running causal prefix over this block (carry = last col)
                nc.vector.tensor_tensor_scan(
                    qs[:, c0:c0 + CB],
                    qsum[:, c0:c0 + CB],
                    scl[:, c0:c0 + CB],
                    0.0 if b == 0 else qs[:, c0 - 1:c0],
                    op0=ALU.add,
                    op1=ALU.bypass,
                )
                nc.vector.tensor_mul(
                    h[:, c0:c0 + CB], qs[:, c0:c0 + CB], scl[:, c0:c0 + CB]
                )
                nc.scalar.activation(
                    hb[:, c0:c0 + CB], h[:, c0:c0 + CB], AF.Relu,
                    bias=b1s[:, :1],
                )

            odst = out_d.rearrange("(dt p) t -> p dt t", p=128)

            def stage_gate(b):
                t0 = b * TB
                c0 = b * CB
                for di in range(NDT):
                    gp = psum_g.tile([128, CB], f32, tag="g", name="gp")
                    nc.tensor.matmul(
                        gp[:],
                        w2s[:, di * 128:(di + 1) * 128],
                        hb[:, c0:c0 + CB],
                        start=True,
                        stop=True,
                    )
                    nc.scalar.activation(
                        gs[:, di, c0:c0 + CB], gp[:], AF.Sigmoid,
                        bias=b2s[:, di:di + 1],
                    )
                    # gate-multiply in place in SBUF: DVE d0/d1, GpSimd
                    # d2/d3; fused pair stores (semaphore budget) split
                    # over two hardware rings (ACT ring for DVE's tiles,
                    # sync ring FIFO for GpSimd's)
                    xv = xt[:, di, t0:t0 + TB].rearrange(
                        "p (c j) -> p c j", j=CS
                    )
                    gv = (
                        gs[:, di, c0:c0 + CB]
                        .unsqueeze(2)
                        .broadcast_to([128, CB, CS])
                    )
                    # output view tagged f32r: the BIR verifier rejects any
                    # non-f32r writer into a buffer an FP32r matmult reads
                    xo = xv.bitcast(f32r)
                    if di < 2:
                        nc.vector.tensor_tensor(xo, xv, gv, op=ALU.mult)
                    else:
                        nc.gpsimd.tensor_tensor(xo, xv, gv, op=ALU.mult)
                nc.scalar.dma_start(
                    odst[:, 0:2, t0:t0 + TB], xt[:, 0:2, t0:t0 + TB]
                )
                nc.sync.dma_start(
                    odst[:, 2:4, t0:t0 + TB], xt[:, 2:4, t0:t0 + TB]
                )

            # Macro-steps of 2 blocks. Per-engine program order within a
            # step: PE runs the (small) g-matmuls of the lag-2 blocks
            # FIRST, then the 16-matmul q-burst; DVE runs the lag-2 gate
            # multiplies first, then the reduce/scan of the current pair.
            # No engine's program ever waits on same-step upstream results,
            # so the whole pipeline free-runs ahead of the DMA rings.
            for m in range(0, NB + 2, 2):
                for bb in (m - 2, m - 1):
                    if 0 <= bb < NB:
                        stage_gate(bb)
                if m < NB:
                    stage_q2(m)
                for bb in (m, m + 1):
                    if 0 <= bb < NB:
                        stage_prefix(bb)
    # run_bass_via_pjrt serializes nc.m as-is; Bacc defers register
    # allocation and TRN2 sync-wait legalization to finalize(), so it must
    # run here or walrus rejects the BIR.
    nc.finalize()
    return nc


def _host_inputs(x, w1, b1, w2, b2, chunksize):
    x = np.ascontiguousarray(np.asarray(x, dtype=np.float32))
    w1 = np.asarray(w1, dtype=np.float32)
    b1 = np.ascontiguousarray(np.asarray(b1, dtype=np.float32))
    w2 = np.asarray(w2, dtype=np.float32)
    b2 = np.ascontiguousarray(np.asarray(b2, dtype=np.float32))
    cs = int(chunksize)
    assert cs == CS and x.shape == (B, D, T), (cs, x.shape)
    import ml_dtypes

    w1t = np.ascontiguousarray(w1.T)                      # [D, DH]
    w2t = np.ascontiguousarray(w2.T.astype(ml_dtypes.bfloat16))  # [DH, D]
    scale = np.broadcast_to(
        1.0 / (CS * np.arange(1, TC + 1, dtype=np.float32)), (DH, TC)
    )
    scale = np.ascontiguousarray(scale)
    shared = dict(w1t=w1t, b1=b1, w2t=w2t, b2=b2, scale=scale)
    return x, shared


def kernel(x, w1, b1, w2, b2, chunksize):
    global _compiled_nc
    from concourse.bass_utils import run_bass_kernel_spmd

    x, shared = _host_inputs(x, w1, b1, w2, b2, chunksize)
    if _compiled_nc is None:
        _compiled_nc = build_nc()
    in_maps = [
        {"x": np.ascontiguousarray(x[i]), **shared} for i in range(NCORES)
    ]
    res = run_bass_kernel_spmd(_compiled_nc, in_maps, list(range(NCORES)))
    out = np.stack([res.results[i]["out"] for i in range(NCORES)], axis=0)
    return out


# revision 23
# speedup vs baseline: 1.0294x; 1.0294x over previous
"""Trainium2 Bass kernel for nn_CausalSE: causal cumulative-average pooling
+ squeeze-excite gating, data-parallel over batch (one NeuronCore per batch
element).

Reference math per batch element (D=512, T=8192, chunk=16, Tc=512):
    avg    = cumsum(x, t) / (t+1)
    pooled = avg[:, 15::16]                          # [D, Tc]
    h      = relu(w1 @ pooled + b1)                  # [64, Tc]
    g      = sigmoid(w2 @ h + b2)                    # [D, Tc]
    out    = repeat(g, 16, t)[:, :T] * x

The kernel is DMA-bound: 16 MB in + 16 MB out per core at ~435 GB/s DDR is
a ~77 us floor, so everything is scheduled around keeping the DMA rings
saturated end-to-end. Loads all enqueue first on the sync ring (full-rate
prefetch, stores drain behind them FIFO); the gate-multiply outputs feed
two rings (sync + ACT) so neither ring's write path caps the tail.

Compute per 1024-col t-block is sized well under the 4.8 us/block DMA
period: the chunk-sums go through the Tensor engine as q = w1 @ x in
float32r (single-pass fp32 matmul, 1 cycle/row for N>=256 — no cast
needed) so DVE only chunk-reduces the [64, t] PSUM result (16x less
reduce work than reducing x), then scans the causal prefix. The four
in-place gate multiplies split DVE (d0,d1) / GpSimd (d2,d3). x stays
resident in SBUF; total HBM traffic is the 32 MB minimum.
"""

import sys

for _p in ("/opt/trn_rl_repo",):
    if _p not in sys.path:
        sys.path.insert(0, _p)

import numpy as np

B, D, T = 8, 512, 8192
DH = 64          # bottleneck dim = D // 8
CS = 16          # chunksize
TC = T // CS     # 512 chunks
NCORES = 8
NDT = D // 128   # 4 partition tiles of x / out
NB = 8           # t-blocks in the causal pipeline
TB = T // NB     # 1024 cols per block (4 KB DMA rows)
CB = TB // CS    # 64 chunks per block
TH = TB // 2     # 512-col halves: matmul N>=256 keeps float32r at 1 cy/row
CH = TH // CS    # 32 chunks per half

_compiled_nc = None


def build_nc():
    import concourse.tile as tile
    from concourse import bacc, mybir

    f32 = mybir.dt.float32
    f32r = mybir.dt.float32r
    bf16 = mybir.dt.bfloat16
    AF = mybir.ActivationFunctionType
    ALU = mybir.AluOpType
    AX = mybir.AxisListType

    # Bacc (not plain Bass): its finalize() runs the TRN2 sync-wait
    # legalization (move_matmul_waits_to_ldweights / event-semaphore
    # splitting) that walrus codegen requires.
    nc = bacc.Bacc("TRN2", target_bir_lowering=False)
    x_d = nc.declare_dram_parameter("x", [D, T], f32, isOutput=False)
    # weights pre-packed host-side into 3 descriptor-efficient buffers:
    # wb = [w1 | b2] per partition, sb = [scale | b1], w2 on its own
    wb_d = nc.declare_dram_parameter("wb", [128, NDT * DH + NDT], f32,
                                     isOutput=False)
    w2t_d = nc.declare_dram_parameter("w2t", [DH, D], bf16, isOutput=False)
    sb_d = nc.declare_dram_parameter("sb", [DH, TC + 1], f32, isOutput=False)
    out_d = nc.declare_dram_parameter("out", [D, T], f32, isOutput=True)

    with tile.TileContext(nc) as tc:
        with (
            tc.tile_pool(name="xres", bufs=1) as xres,
            tc.tile_pool(name="small", bufs=1) as small,
            tc.tile_pool(name="psum_y", bufs=4, space="PSUM") as psum_y,
            tc.tile_pool(name="psum_g", bufs=4, space="PSUM") as psum_g,
        ):
            xt = xres.tile([128, NDT, T], f32, tag="x")
            gs = small.tile([128, NDT, TC], f32, tag="gs")
            wb = small.tile([128, NDT * DH + NDT], f32, tag="wb")
            w2s = small.tile([DH, D], bf16, tag="w2")
            sb = small.tile([DH, TC + 1], f32, tag="sb")
            qsum = small.tile([DH, TC], f32, tag="qsum")  # chunk sums of w1@x
            qs = small.tile([DH, TC], f32, tag="qs")      # causal prefix
            h = small.tile([DH, TC], f32, tag="h")
            hb = small.tile([DH, TC], bf16, tag="hb")     # relu(h) for g-mm
            w1s = wb[:, 0:NDT * DH].rearrange("p (k o) -> p k o", o=DH)
            b2s = wb[:, NDT * DH:NDT * DH + NDT]
            scl = sb[:, 0:TC]
            b1s = sb[:, TC:TC + 1]

            # -- replicated weights / constants, on the ACT queue so the
            # sync ring's 16 MB x prefetch starts at first byte. Only 8
            # DMA-completion semaphores exist globally and each dma_start
            # waits for its semaphore's previous user to COMPLETE, so the
            # weights go as 3 host-packed descriptor-efficient DMAs —
            # fragmented weight loads take 10+ us to drain and stall the
            # x loads that recycle their semaphores. (wb dest tagged f32r:
            # the BIR verifier requires FP32r matmult operands from an
            # f32r-typed producer; b2's bias use is dtype-agnostic bits.)
            nc.scalar.dma_start(wb[:].bitcast(f32r), wb_d[:].bitcast(f32r))
            nc.scalar.dma_start(w2s[:], w2t_d[:])
            nc.scalar.dma_start(sb[:], sb_d[:])

            # All loads enqueue up front on the sync ring: the ring drains
            # FIFO, so loads prefetch at the full rate and the stores
            # appended later fill the ring's tail without starving it.
            # One fused 2 MB DMA per block — separate per-dtile loads issue
            # ~2 us apart (DMA-semaphore recycle latency), which starves
            # the ring and pushes the last block's arrival past 70 us.
            xsrc = x_d.rearrange("(dt p) t -> p dt t", p=128)
            for b in range(NB):
                t0 = b * TB
                nc.sync.dma_start(
                    xt[:, :, t0:t0 + TB].bitcast(f32r),
                    xsrc[:, :, t0:t0 + TB].bitcast(f32r),
                )

            # Causal pipeline, software-pipelined with a 2-block skew so no
            # in-order engine ever waits on same-block results: PE runs
            # q-mm(b) before g-mm(b-2), DVE runs reduce/scan(b) before the
            # gate multiplies of block b-2. Compute then runs ahead of the
            # DMA rings and the store tail is pure ring drain.
            yps = {}

            def stage_q2(b0):
                # y = w1 @ x on PE in float32r for blocks b0, b0+1 as one
                # 16-matmul burst, ki-outer so the stationary only changes
                # 4 times and the PE pipeline stays hot (p-state ramp)
                blks = [b for b in (b0, b0 + 1) if b < NB]
                for b in blks:
                    yps[b] = [
                        psum_y.tile([DH, TH], f32, tag="y", name=f"yp{b}_{hh}")
                        for hh in range(2)
                    ]
                for ki in range(NDT):
                    for b in blks:
                        t0 = b * TB
                        for hh in range(2):
                            nc.tensor.matmul(
                                yps[b][hh][:],
                                w1s[:, ki, :].bitcast(f32r),
                                xt[:, ki, t0 + hh * TH:t0 + (hh + 1) * TH]
                                .bitcast(f32r),
                                start=(ki == 0),
                                stop=(ki == NDT - 1),
                            )

            def stage_prefix(b):
                c0 = b * CB
                yp = yps.pop(b)
                # chunk sums of y: 16x less DVE reduce work than on x
                for hh in range(2):
                    nc.vector.reduce_sum(
                        qsum[:, c0 + hh * CH:c0 + (hh + 1) * CH],
                        yp[hh][:].rearrange("p (c j) -> p c j", j=CS),
                        axis=AX.X,
                    )
                # running causal prefix over this block (carry = last col)
                nc.vector.tensor_tensor_scan(
                    qs[:, c0:c0 + CB],
                    qsum[:, c0:c0 + CB],
                    scl[:, c0:c0 + CB],
                    0.0 if b == 0 else qs[:, c0 - 1:c0],
                    op0=ALU.add,
                    op1=ALU.bypass,
                )
                nc.vector.tensor_mul(
                    h[:, c0:c0 + CB], qs[:, c0:c0 + CB], scl[:, c0:c0 + CB]
                )
                nc.scalar.activation(
                    hb[:, c0:c0 + CB], h[:, c0:c0 + CB], AF.Relu,
                    bias=b1s[:, :1],
                )

            odst = out_d.rearrange("(dt p) t -> p dt t", p=128)

            def stage_gate(b):
                t0 = b * TB
                c0 = b * CB
                for di in range(NDT):
                    gp = psum_g.tile([128, CB], f32, tag="g", name="gp")
                    nc.tensor.matmul(
                        gp[:],
                        w2s[:, di * 128:(di + 1) * 128],
                        hb[:, c0:c0 + CB],
                        start=True,
                        stop=True,
                    )
                    nc.scalar.activation(
                        gs[:, di, c0:c0 + CB], gp[:], AF.Sigmoid,
                        bias=b2s[:, di:di + 1],
                    )
                    # gate-multiply in place in SBUF: DVE d0/d1, GpSimd
                    # d2/d3; fused pair stores (semaphore budget) split
                    # over two hardware rings (ACT ring for DVE's tiles,
                    # sync ring FIFO for GpSimd's)
                    xv = xt[:, di, t0:t0 + TB].rearrange(
                        "p (c j) -> p c j", j=CS
                    )
                    gv = (
                        gs[:, di, c0:c0 + CB]
                        .unsqueeze(2)
                        .broadcast_to([128, CB, CS])
                    )
                    # output view tagged f32r: the BIR verifier rejects any
                    # non-f32r writer into a buffer an FP32r matmult reads
                    xo = xv.bitcast(f32r)
                    if di < 2:
                        nc.vector.tensor_tensor(xo, xv, gv, op=ALU.mult)
                        st_eng = nc.scalar
                    else:
                        nc.gpsimd.tensor_tensor(xo, xv, gv, op=ALU.mult)
                        st_eng = nc.sync
                    st_eng.dma_start(
                        odst[:, di, t0:t0 + TB], xt[:, di, t0:t0 + TB]
                    )

            # Macro-steps of 2 blocks. Per-engine program order within a
            # step: PE runs the (small) g-matmuls of the lag-2 blocks
            # FIRST, then the 16-matmul q-burst; DVE runs the lag-2 gate
            # multiplies first, then the reduce/scan of the current pair.
            # No engine's program ever waits on same-step upstream results,
            # so the whole pipeline free-runs ahead of the DMA rings.
            for m in range(0, NB + 2, 2):
                for bb in (m - 2, m - 1):
                    if 0 <= bb < NB:
                        stage_gate(bb)
                if m < NB:
                    stage_q2(m)
                for bb in (m, m + 1):
                    if 0 <= bb < NB:
                        stage_prefix(bb)
    # run_bass_via_pjrt serializes nc.m as-is; Bacc defers register
    # allocation and TRN2 sync-wait legalization to finalize(), so it must
    # run here or walrus rejects the BIR.
    nc.finalize()
    return nc


def _host_inputs(x, w1, b1, w2, b2, chunksize):
    x = np.ascontiguousarray(np.asarray(x, dtype=np.float32))
    w1 = np.asarray(w1, dtype=np.float32)
    b1 = np.ascontiguousarray(np.asarray(b1, dtype=np.float32))
    w2 = np.asarray(w2, dtype=np.float32)
    b2 = np.ascontiguousarray(np.asarray(b2, dtype=np.float32))
    cs = int(chunksize)
    assert cs == CS and x.shape == (B, D, T), (cs, x.shape)
    import ml_dtypes

    # wb[p, :] = [w1 stationary tiles | b2], one 1 KB+ descriptor per
    # partition; sb = [scale | b1] on 64 partitions
    w1p = w1.T.reshape(NDT, 128, DH).transpose(1, 0, 2).reshape(128, NDT * DH)
    b2p = b2.reshape(NDT, 128).T                          # [128, NDT]
    wb = np.ascontiguousarray(np.concatenate([w1p, b2p], axis=1))
    w2t = np.ascontiguousarray(w2.T.astype(ml_dtypes.bfloat16))  # [DH, D]
    scale = np.broadcast_to(
        1.0 / (CS * np.arange(1, TC + 1, dtype=np.float32)), (DH, TC)
    )
    sb = np.ascontiguousarray(
        np.concatenate([scale, np.broadcast_to(b1[:, None], (DH, 1))], axis=1)
    )
    shared = dict(wb=wb, w2t=w2t, sb=sb)
    return x, shared


def kernel(x, w1, b1, w2, b2, chunksize):
    global _compiled_nc
    from concourse.bass_utils import run_bass_kernel_spmd

    x, shared = _host_inputs(x, w1, b1, w2, b2, chunksize)
    if _compiled_nc is None:
        _compiled_nc = build_nc()
    in_maps = [
        {"x": np.ascontiguousarray(x[i]), **shared} for i in range(NCORES)
    ]
    res = run_bass_kernel_spmd(_compiled_nc, in_maps, list(range(NCORES)))
    out = np.stack([res.results[i]["out"] for i in range(NCORES)], axis=0)
    return out


# revision 24
# speedup vs baseline: 1.1319x; 1.0995x over previous
"""Trainium2 Bass kernel for nn_CausalSE: causal cumulative-average pooling
+ squeeze-excite gating, data-parallel over batch (one NeuronCore per batch
element).

Reference math per batch element (D=512, T=8192, chunk=16, Tc=512):
    avg    = cumsum(x, t) / (t+1)
    pooled = avg[:, 15::16]                          # [D, Tc]
    h      = relu(w1 @ pooled + b1)                  # [64, Tc]
    g      = sigmoid(w2 @ h + b2)                    # [D, Tc]
    out    = repeat(g, 16, t)[:, :T] * x

The kernel is DMA-bound: 16 MB in + 16 MB out per core at ~435 GB/s DDR is
a ~77 us floor, so everything is scheduled around keeping the DMA rings
saturated end-to-end. Loads all enqueue first on the sync ring (full-rate
prefetch, stores drain behind them FIFO); the gate-multiply outputs feed
two rings (sync + ACT) so neither ring's write path caps the tail.

Compute per 1024-col t-block is sized well under the 4.8 us/block DMA
period: the chunk-sums go through the Tensor engine as q = w1 @ x in
float32r (single-pass fp32 matmul, 1 cycle/row for N>=256 — no cast
needed) so DVE only chunk-reduces the [64, t] PSUM result (16x less
reduce work than reducing x), then scans the causal prefix. The four
in-place gate multiplies split DVE (d0,d1) / GpSimd (d2,d3). x stays
resident in SBUF; total HBM traffic is the 32 MB minimum.
"""

import sys

for _p in ("/opt/trn_rl_repo",):
    if _p not in sys.path:
        sys.path.insert(0, _p)

import numpy as np

B, D, T = 8, 512, 8192
DH = 64          # bottleneck dim = D // 8
CS = 16          # chunksize
TC = T // CS     # 512 chunks
NCORES = 8
NDT = D // 128   # 4 partition tiles of x / out
NB = 8           # t-blocks in the causal pipeline
TB = T // NB     # 1024 cols per block (4 KB DMA rows)
CB = TB // CS    # 64 chunks per block
TH = TB // 2     # 512-col halves: matmul N>=256 keeps float32r at 1 cy/row
CH = TH // CS    # 32 chunks per half

_compiled_nc = None


def build_nc():
    import concourse.tile as tile
    from concourse import bacc, mybir

    f32 = mybir.dt.float32
    f32r = mybir.dt.float32r
    bf16 = mybir.dt.bfloat16
    AF = mybir.ActivationFunctionType
    ALU = mybir.AluOpType
    AX = mybir.AxisListType

    # Bacc (not plain Bass): its finalize() runs the TRN2 sync-wait
    # legalization (move_matmul_waits_to_ldweights / event-semaphore
    # splitting) that walrus codegen requires.
    nc = bacc.Bacc("TRN2", target_bir_lowering=False)
    x_d = nc.declare_dram_parameter("x", [D, T], f32, isOutput=False)
    # weights pre-packed host-side into 3 descriptor-efficient buffers:
    # wb = [w1 | b2] per partition, sb = [scale | b1], w2 on its own
    wb_d = nc.declare_dram_parameter("wb", [128, NDT * DH + NDT], f32,
                                     isOutput=False)
    w2t_d = nc.declare_dram_parameter("w2t", [DH, D], bf16, isOutput=False)
    sb_d = nc.declare_dram_parameter("sb", [DH, TC + 1], f32, isOutput=False)
    out_d = nc.declare_dram_parameter("out", [D, T], f32, isOutput=True)

    with tile.TileContext(nc) as tc:
        with (
            tc.tile_pool(name="xres", bufs=1) as xres,
            tc.tile_pool(name="small", bufs=1) as small,
            tc.tile_pool(name="psum_y", bufs=4, space="PSUM") as psum_y,
            tc.tile_pool(name="psum_g", bufs=4, space="PSUM") as psum_g,
        ):
            xt = xres.tile([128, NDT, T], f32, tag="x")
            gs = small.tile([128, NDT, TC], bf16, tag="gs")
            wb = small.tile([128, NDT * DH + NDT], f32, tag="wb")
            w2s = small.tile([DH, D], bf16, tag="w2")
            sb = small.tile([DH, TC + 1], f32, tag="sb")
            qsum = small.tile([DH, TC], f32, tag="qsum")  # chunk sums of w1@x
            qs = small.tile([DH, TC], f32, tag="qs")      # causal prefix
            h = small.tile([DH, TC], f32, tag="h")
            hb = small.tile([DH, TC], bf16, tag="hb")     # relu(h) for g-mm
            w1s = wb[:, 0:NDT * DH].rearrange("p (k o) -> p k o", o=DH)
            b2s = wb[:, NDT * DH:NDT * DH + NDT]
            scl = sb[:, 0:TC]
            b1s = sb[:, TC:TC + 1]

            # -- replicated weights / constants, on the ACT queue so the
            # sync ring's 16 MB x prefetch starts at first byte. Only 8
            # DMA-completion semaphores exist globally and each dma_start
            # waits for its semaphore's previous user to COMPLETE, so the
            # weights go as 3 host-packed descriptor-efficient DMAs —
            # fragmented weight loads take 10+ us to drain and stall the
            # x loads that recycle their semaphores. (wb dest tagged f32r:
            # the BIR verifier requires FP32r matmult operands from an
            # f32r-typed producer; b2's bias use is dtype-agnostic bits.)
            nc.scalar.dma_start(wb[:].bitcast(f32r), wb_d[:].bitcast(f32r))
            nc.scalar.dma_start(w2s[:], w2t_d[:])
            nc.scalar.dma_start(sb[:], sb_d[:])

            # All loads enqueue up front on the sync ring: the ring drains
            # FIFO, so loads prefetch at the full rate and the stores
            # appended later fill the ring's tail without starving it.
            # One fused 2 MB DMA per block — separate per-dtile loads issue
            # ~2 us apart (DMA-semaphore recycle latency), which starves
            # the ring and pushes the last block's arrival past 70 us.
            xsrc = x_d.rearrange("(dt p) t -> p dt t", p=128)
            for b in range(NB):
                t0 = b * TB
                nc.sync.dma_start(
                    xt[:, :, t0:t0 + TB].bitcast(f32r),
                    xsrc[:, :, t0:t0 + TB].bitcast(f32r),
                )

            # Causal pipeline, software-pipelined with a 2-block skew so no
            # in-order engine ever waits on same-block results: PE runs
            # q-mm(b) before g-mm(b-2), DVE runs reduce/scan(b) before the
            # gate multiplies of block b-2. Compute then runs ahead of the
            # DMA rings and the store tail is pure ring drain.
            yps = {}

            def stage_q2(b0):
                # y = w1 @ x on PE in float32r for blocks b0, b0+1 as one
                # 16-matmul burst, ki-outer so the stationary only changes
                # 4 times and the PE pipeline stays hot (p-state ramp)
                blks = [b for b in (b0, b0 + 1) if b < NB]
                for b in blks:
                    yps[b] = [
                        psum_y.tile([DH, TH], f32, tag="y", name=f"yp{b}_{hh}")
                        for hh in range(2)
                    ]
                for ki in range(NDT):
                    for b in blks:
                        t0 = b * TB
                        for hh in range(2):
                            nc.tensor.matmul(
                                yps[b][hh][:],
                                w1s[:, ki, :].bitcast(f32r),
                                xt[:, ki, t0 + hh * TH:t0 + (hh + 1) * TH]
                                .bitcast(f32r),
                                start=(ki == 0),
                                stop=(ki == NDT - 1),
                            )

            def stage_prefix(b):
                c0 = b * CB
                yp = yps.pop(b)
                # chunk sums of y: 16x less DVE reduce work than on x
                for hh in range(2):
                    nc.vector.reduce_sum(
                        qsum[:, c0 + hh * CH:c0 + (hh + 1) * CH],
                        yp[hh][:].rearrange("p (c j) -> p c j", j=CS),
                        axis=AX.X,
                    )
                # running causal prefix over this block (carry = last col)
                nc.vector.tensor_tensor_scan(
                    qs[:, c0:c0 + CB],
                    qsum[:, c0:c0 + CB],
                    scl[:, c0:c0 + CB],
                    0.0 if b == 0 else qs[:, c0 - 1:c0],
                    op0=ALU.add,
                    op1=ALU.bypass,
                )
                nc.vector.tensor_mul(
                    h[:, c0:c0 + CB], qs[:, c0:c0 + CB], scl[:, c0:c0 + CB]
                )
                nc.scalar.activation(
                    hb[:, c0:c0 + CB], h[:, c0:c0 + CB], AF.Relu,
                    bias=b1s[:, :1],
                )

            odst = out_d.rearrange("(dt p) t -> p dt t", p=128)

            def stage_gate(b):
                t0 = b * TB
                c0 = b * CB
                for di in range(NDT):
                    gp = psum_g.tile([128, CB], f32, tag="g", name="gp")
                    nc.tensor.matmul(
                        gp[:],
                        w2s[:, di * 128:(di + 1) * 128],
                        hb[:, c0:c0 + CB],
                        start=True,
                        stop=True,
                    )
                    nc.scalar.activation(
                        gs[:, di, c0:c0 + CB], gp[:], AF.Sigmoid,
                        bias=b2s[:, di:di + 1],
                    )
                    # gate-multiply in place in SBUF: DVE d0/d1, GpSimd
                    # d2/d3; fused pair stores (semaphore budget) split
                    # over two hardware rings (ACT ring for DVE's tiles,
                    # sync ring FIFO for GpSimd's)
                    xv = xt[:, di, t0:t0 + TB].rearrange(
                        "p (c j) -> p c j", j=CS
                    )
                    gv = (
                        gs[:, di, c0:c0 + CB]
                        .unsqueeze(2)
                        .broadcast_to([128, CB, CS])
                    )
                    # output view tagged f32r: the BIR verifier rejects any
                    # non-f32r writer into a buffer an FP32r matmult reads
                    xo = xv.bitcast(f32r)
                    if di < 2:
                        nc.vector.tensor_tensor(xo, xv, gv, op=ALU.mult)
                        st_eng = nc.scalar
                    else:
                        nc.gpsimd.tensor_tensor(xo, xv, gv, op=ALU.mult)
                        st_eng = nc.sync
                    st_eng.dma_start(
                        odst[:, di, t0:t0 + TB], xt[:, di, t0:t0 + TB]
                    )

            # Macro-steps of 2 blocks. Per-engine program order within a
            # step: PE runs the (small) g-matmuls of the lag-2 blocks
            # FIRST, then the 16-matmul q-burst; DVE runs the lag-2 gate
            # multiplies first, then the reduce/scan of the current pair.
            # No engine's program ever waits on same-step upstream results,
            # so the whole pipeline free-runs ahead of the DMA rings.
            for m in range(0, NB + 2, 2):
                for bb in (m - 2, m - 1):
                    if 0 <= bb < NB:
                        stage_gate(bb)
                if m < NB:
                    stage_q2(m)
                for bb in (m, m + 1):
                    if 0 <= bb < NB:
                        stage_prefix(bb)
    # run_bass_via_pjrt serializes nc.m as-is; Bacc defers register
    # allocation and TRN2 sync-wait legalization to finalize(), so it must
    # run here or walrus rejects the BIR.
    nc.finalize()
    return nc


def _host_inputs(x, w1, b1, w2, b2, chunksize):
    x = np.ascontiguousarray(np.asarray(x, dtype=np.float32))
    w1 = np.asarray(w1, dtype=np.float32)
    b1 = np.ascontiguousarray(np.asarray(b1, dtype=np.float32))
    w2 = np.asarray(w2, dtype=np.float32)
    b2 = np.ascontiguousarray(np.asarray(b2, dtype=np.float32))
    cs = int(chunksize)
    assert cs == CS and x.shape == (B, D, T), (cs, x.shape)
    import ml_dtypes

    # wb[p, :] = [w1 stationary tiles | b2], one 1 KB+ descriptor per
    # partition; sb = [scale | b1] on 64 partitions
    w1p = w1.T.reshape(NDT, 128, DH).transpose(1, 0, 2).reshape(128, NDT * DH)
    b2p = b2.reshape(NDT, 128).T                          # [128, NDT]
    wb = np.ascontiguousarray(np.concatenate([w1p, b2p], axis=1))
    w2t = np.ascontiguousarray(w2.T.astype(ml_dtypes.bfloat16))  # [DH, D]
    scale = np.broadcast_to(
        1.0 / (CS * np.arange(1, TC + 1, dtype=np.float32)), (DH, TC)
    )
    sb = np.ascontiguousarray(
        np.concatenate([scale, np.broadcast_to(b1[:, None], (DH, 1))], axis=1)
    )
    shared = dict(wb=wb, w2t=w2t, sb=sb)
    return x, shared


def kernel(x, w1, b1, w2, b2, chunksize):
    global _compiled_nc
    from concourse.bass_utils import run_bass_kernel_spmd

    x, shared = _host_inputs(x, w1, b1, w2, b2, chunksize)
    if _compiled_nc is None:
        _compiled_nc = build_nc()
    in_maps = [
        {"x": np.ascontiguousarray(x[i]), **shared} for i in range(NCORES)
    ]
    res = run_bass_kernel_spmd(_compiled_nc, in_maps, list(range(NCORES)))
    out = np.stack([res.results[i]["out"] for i in range(NCORES)], axis=0)
    return out
